# revision 3
# baseline (speedup 1.0000x reference)
"""CrossEntropyLossWithProb on 8 trn2 NeuronCores -- v3.

loss = -mean(log(max(probs[i, labels[i]], 1e-8)))  over i in [0, 8192)

Row-sharded across 8 cores (1024 rows each).  All 1024 rows of a core
are fetched by 4 PREPARE_ONLY dma_gather windows fired by trigger_dma:

  - Window a covers rows [256a, 256a+256).  Within the 256-row slab
    (flat f32 range [256a*32000, ...+8.192M)), row j's labeled element
    sits in 1KB-aligned block idx16 = 125*j + (L>>8) (32000 = 125*256,
    so block indices stay < 32000 < int16 max), at within-window offset
    L&255.  One SWDGE prep covers 256 rows (994+0.34*256 ns) vs the
    baseline's two 128-offset indirect DMAs (2x1038 ns) -- the HW honors
    only one dynamic offset per offset-AP partition, so plain indirect
    DMAs cannot batch more than 128 rows (probed: multi-column offset
    APs degrade to consecutive-run fetches).
  - PREPARE_ONLY + trigger_dma: a triggered transfer skips the 650 ns
    DGE->DMA delay that a normal Pool SWDGE transfer pays.  All 4 preps
    are issued first (they fill Pool's 4-deep exec queue, keeping the
    Pool engine back-to-back), then 4 wait+trigger pairs fire each
    window's transfer as soon as its descriptors are committed.
  - The within-window select folds clamp+select+reduce into ONE DVE
    scalar_tensor_tensor per output column: accum_out = sum((g max 1e-8)
    * M01) = max(p_row, 1e-8) exactly, with M01 an uploaded bf16 one-hot
    mask (bf16 so its upload clears DMA_ENGINES before the first window
    transfer; 0.0/1.0 are exact in bf16).  (tensor_tensor_reduce would
    do a whole window in one op but that raw-ISA instruction dies on
    this toolchain -- probed; TensorScalarPtr is standard BIR and runs.)
  - One ln over the 8 selected values per partition writes acc directly
    (no accumulate-read); the host sums the 8 result columns.
  - The output leaves via a 5th PREPARE_ONLY dma_scatter_add prepped on
    Pool's idle tail and fired by a final trigger_dma after the ln (the
    scatter reads acc at trigger time, skipping both the 625 ns HWDGE
    gen and the 650 ns DGE delay of an SP DMA).
  - The gather-idx upload is hoisted ahead of the framework's const-init
    barrier (it alone gates the preps); mask/scatter-idx uploads follow
    post-barrier, their latency hidden under the preps.

  SP  : dma idxg[128,64]i16 (pre-barrier), M01[128,8,256]bf16,
        sidx[128,8]i16
  PL  : memset acc=0; 4 gather preps; 4 trigger_dma; scatter-prep;
        final trigger; tail dma_reset+sem_clear
  DVE : 8 x stt(max 1e-8, mult-mask, accum) -> p_t[128,8]
  ACT : ln(p_t) -> acc[:,0:8]
"""

import numpy as np

import concourse.bacc as bacc
import concourse.bass as bass
import concourse.mybir as mybir
from concourse.bass import compact_to_ranges

B, V = 8192, 32000
N_CORES = 8
BS = B // N_CORES          # 1024 rows per core
W = 256                    # rows per gather window
NW = 4                     # gather windows cover all rows
CLIP = 1e-8

_cached_nc = None


def build_nc(detect_races=False):
    global _cached_nc
    if _cached_nc is not None and not detect_races:
        return _cached_nc

    nc = bacc.Bacc("TRN2", target_bir_lowering=False, debug=False,
                   num_devices=N_CORES,
                   detect_race_conditions=detect_races)
    probs = nc.dram_tensor("probs", [BS, V], mybir.dt.float32,
                           kind="ExternalInput")
    idxg = nc.dram_tensor("idxg", [128, NW * 16], mybir.dt.int16,
                          kind="ExternalInput")
    m01 = nc.dram_tensor("m01", [128, NW * 2 * W], mybir.dt.bfloat16,
                         kind="ExternalInput")
    sidx = nc.dram_tensor("sidx", [128, 8], mybir.dt.int16,
                          kind="ExternalInput")
    out = nc.dram_tensor("out", [128, 64], mybir.dt.float32,
                         kind="ExternalOutput")

    with (
        nc.sbuf_tensor("idxg_t", [128, NW * 16], mybir.dt.int16) as idxg_t,
        nc.sbuf_tensor("m01_t", [128, NW * 2, W], mybir.dt.bfloat16) as m01_t,
        nc.sbuf_tensor("sidx_t", [128, 8], mybir.dt.int16) as sidx_t,
        nc.sbuf_tensor("g_t", [128, NW * 2, W], mybir.dt.float32) as g_t,
        nc.sbuf_tensor("tt_t", [128, W], mybir.dt.float32) as tt_t,
        nc.sbuf_tensor("p_t", [128, NW * 2], mybir.dt.float32) as p_t,
        nc.sbuf_tensor("acc_t", [128, 1, 64], mybir.dt.float32) as acc_t,
        nc.semaphore("s_idx") as s_idx,
        nc.semaphore("s_m2") as s_m2,
        nc.semaphore("s_sx") as s_sx,
        nc.semaphore("s_gp") as s_gp,
        nc.semaphore("s_g") as s_g,
        nc.semaphore("s_v") as s_v,
        nc.semaphore("s_act") as s_act,
        nc.semaphore("s_prep") as s_prep,
        nc.semaphore("s_dma") as s_dma,
    ):
        # SP: idxg DMA is hoisted pre-barrier below (it alone gates the
        # gather preps).  The others issue post-barrier; their latency
        # hides under the preps.
        nc.sync.dma_start(idxg_t[:], idxg.ap()).then_inc(s_idx, 16)
        nc.sync.dma_start(m01_t[:], m01.ap()).then_inc(s_m2, 16)
        nc.sync.dma_start(sidx_t[:], sidx.ap()).then_inc(s_sx, 16)

        # PL preamble.
        nc.gpsimd.memset(acc_t[:], 0.0)
        r_w = nc.gpsimd.to_reg(W)
        r_128 = nc.gpsimd.to_reg(128)

        # PL: 4 window gather preps back-to-back (4-deep exec queue),
        # then 4 wait+trigger pairs -- each window's transfer fires the
        # moment its descriptors are committed, skipping the DGE delay.
        nc.gpsimd.wait_ge(s_idx, 16)
        for a in range(NW):
            in_ap = bass.AP(probs, a * W * V, [[W, V], [1, W]])
            nc.gpsimd.dma_gather(
                out_ap=g_t[:, 2 * a:2 * a + 2, :], in_ap=in_ap,
                idxs_ap=idxg_t[:, 16 * a:16 * a + 16],
                num_idxs=W, num_idxs_reg=r_w, elem_size=W,
                prepare_only=True, sem=s_g,
            ).then_inc(s_gp, 1)
        for a in range(NW):
            nc.gpsimd.wait_ge(s_gp, a + 1)
            nc.gpsimd.trigger_dma(count=1)

        # PL: output scatter prepped on the idle tail, triggered after ln.
        nc.gpsimd.wait_ge(s_sx, 16)
        nc.gpsimd.dma_scatter_add(
            out_ap=out.ap(), in_ap=acc_t[:], idxs_ap=sidx_t[:],
            num_idxs=128, num_idxs_reg=r_128, elem_size=64,
            prepare_only=True, sem=s_dma,
        ).then_inc(s_prep, 1)

        # DVE: one clamp+select+sum per output column:
        # accum = sum((g max 1e-8) * one-hot) = max(p_row, 1e-8).
        nc.vector.wait_ge(s_m2, 16)
        for a in range(NW):
            nc.vector.wait_ge(s_g, 16 * (a + 1))
            for c in (2 * a, 2 * a + 1):
                nc.vector.scalar_tensor_tensor(
                    out=tt_t[:], in0=g_t[:, c, :], scalar=CLIP,
                    in1=m01_t[:, c, :],
                    op0=mybir.AluOpType.max, op1=mybir.AluOpType.mult,
                    accum_out=p_t[:, c:c + 1],
                ).then_inc(s_v, 1)

        # ACT: one ln straight into acc slots (no accumulator read).
        nc.scalar.wait_ge(s_v, 2 * NW)
        nc.scalar.activation(acc_t[:, 0, 0:2 * NW], p_t[:],
                             mybir.ActivationFunctionType.Ln)\
            .then_inc(s_act, 1)

        # PL: fire the prepped output scatter, then clean up.
        nc.gpsimd.wait_ge(s_prep, 1)
        nc.gpsimd.wait_ge(s_act, 1)
        nc.gpsimd.trigger_dma(count=1)
        nc.gpsimd.wait_ge(s_dma, 16)
        sem_ids = sorted(s.num for s in (s_idx, s_m2, s_sx, s_gp, s_g,
                                         s_v, s_act, s_prep, s_dma))
        for sem_range in compact_to_ranges(sem_ids):
            nc.gpsimd.dma_reset(sem_range)
            nc.gpsimd.sem_clear(sem_range)

    # Hoist ONLY the idxg DMA ahead of the framework's const-init barrier
    # in the SP queue: it depends on nothing, so its ~2.3 us latency
    # overlaps the startup barrier and the first gather prep starts early.
    blk = nc.m.functions[0].blocks[0].instructions
    sp = mybir.EngineType.SP
    di = next(i for i, x in enumerate(blk)
              if isinstance(x, mybir.InstDrain) and x.engine == sp)
    ii = next(i for i, x in enumerate(blk)
              if isinstance(x, mybir.InstDMACopy) and x.engine == sp)
    if di < ii:
        inst = blk.pop(ii)
        blk.insert(di, inst)

    nc.compile()
    if not detect_races:
        _cached_nc = nc
    return nc


def _wrap16(vals, pad_cols):
    """[n] int array -> [128, pad_cols] int16 wrapped (value j at
    partition j%16, col j//16), replicated across the 8 groups of 16."""
    n = len(vals)
    a = np.full((16, pad_cols), -1, dtype=np.int16)
    for j in range(n):
        a[j % 16, j // 16] = vals[j]
    return np.tile(a, (8, 1))


def make_in_maps(probs, labels):
    import ml_dtypes
    probs = np.ascontiguousarray(np.asarray(probs), dtype=np.float32)
    labels = np.asarray(labels).astype(np.int64, copy=False)
    assert probs.shape == (B, V) and labels.shape == (B,)

    sidx = _wrap16(np.arange(128), 8)
    in_maps = []
    for c in range(N_CORES):
        lb = labels[c * BS:(c + 1) * BS].astype(np.int32)

        # gather windows: rows a*W..a*W+W-1, idx16 = 125*j + (L>>8)
        idxg = np.empty((128, NW * 16), dtype=np.int16)
        m01 = np.zeros((128, NW * 2, W), dtype=np.float32)
        for a in range(NW):
            lw = lb[a * W:(a + 1) * W]
            idx16 = (125 * np.arange(W, dtype=np.int32)
                     + (lw >> 8)).astype(np.int16)
            idxg[:, 16 * a:16 * a + 16] = _wrap16(idx16, 16)
            j = np.arange(W)
            m01[j % 128, 2 * a + j // 128, lw & 255] = 1.0

        in_maps.append({
            "probs": probs[c * BS:(c + 1) * BS],
            "idxg": idxg,
            "m01": (m01.reshape(128, NW * 2 * W)
                    .astype(ml_dtypes.bfloat16)),
            "sidx": sidx,
        })
    return in_maps


def kernel(probs, labels):
    from concourse.bass_utils import run_bass_kernel_spmd
    nc = build_nc()
    in_maps = make_in_maps(probs, labels)
    res = run_bass_kernel_spmd(nc, in_maps, core_ids=list(range(N_CORES)))
    total = np.float64(0.0)
    for r in res.results:
        total += np.float64(
            r["out"][:, 0:2 * NW].sum(dtype=np.float64))
    return np.array(-total / B, dtype=np.float32)
